# revision 16
# baseline (speedup 1.0000x reference)
"""MoE fused top-k-gating decode kernel for Trainium2 (8 NeuronCores).

Problem: B=32,S=1,H=2048, E=8 experts, I=5632, top_k=2, fp32.
Sharding: expert-parallel - core c owns expert c (w_gate/w_up/w_down[c]),
router weight replicated (rolled per-core so column 0 is the own expert).
Each core computes the full router (softmax + top-2 mask), its expert's
GLU-MLP for all 32 tokens, scales by its combine weight, and returns a
partial [T,H] output; the host sums the 8 partials.

The expert weights stream in bfloat16 (tolerance is 2e-2; measured
end-to-end rel error of bf16 weights+activations is ~3.8e-3), halving
the per-core weight DMA vs fp32: 3*H*I*2B = 69.2 MB @ ~358 GB/s HBM
limit = ~193 us. The router path stays fp32 so the top-2 selection and
combine weights match the reference bit-for-bit in practice.

All weight tensors are pre-blocked on the host into the SBUF-resident
layout ([128 partitions, contiguous (k, cols) runs]) so every weight
DMA is a plain 128-descriptor contiguous copy (8-16 KiB/partition).
Matmuls keep the 32-token activations stationary ([128,32] bf16 lhsT
tiles) and stream the weights as the moving operand (1 cycle/row).

TRN2 allows only ONE sync-wait per instruction; the Bacc layer splits
excess waits into event-semaphore chains. The kernel is arranged so
most PE instructions need at most one new semaphore (operand pairs
arrive in a single DMA, PSUM is read only by the vector engine, junk
transposes absorb DMA ticks), keeping the inserted event chains off
the hot path.
"""

import numpy as np
import ml_dtypes

import concourse.bass as bass
import concourse.bacc as bacc
import concourse.mybir as mybir
import concourse.tile as tile
from concourse.masks import make_identity
from concourse.bass_utils import run_bass_kernel_spmd

B, S, H = 32, 1, 2048
E, I = 8, 5632
T = B * S          # 32 tokens
P = 128            # partitions
NCORES = 8
SWIGLU_SCALE = 1.702

KH = H // P        # 16 contraction chunks over H
KI = I // P        # 44 contraction chunks over I
NW = 512           # moving-dim tile width
ND = H // NW       # 4 down output tiles
XW = T + E         # packed xT+router width (40)

# gate/up column slabs; narrow first slabs so compute starts on the first
# weight bytes landed, narrow last slab to shrink the serial chain after
# the last weight bytes land.
WIDTHS = [NW // 4, NW // 2] + [NW] * 10 + [NW // 4]
assert sum(WIDTHS) == I

F32 = mybir.dt.float32
BF16 = mybir.dt.bfloat16
AX = mybir.AxisListType.X
AF = mybir.ActivationFunctionType
OP = mybir.AluOpType


def _build_nc() -> bass.Bass:
    nc = bacc.Bacc()

    xrw_d = nc.declare_dram_parameter("xrw", [P, KH * XW], F32, isOutput=False)
    xtb_d = nc.declare_dram_parameter("xtb", [P, KH * T], BF16, isOutput=False)
    wg_d = nc.declare_dram_parameter("wg", [P, KH * I], BF16, isOutput=False)
    wu_d = nc.declare_dram_parameter("wu", [P, KH * I], BF16, isOutput=False)
    wd_d = nc.declare_dram_parameter("wd", [P, KI * H], BF16, isOutput=False)
    out_d = nc.declare_dram_parameter("out", [T, H], F32, isOutput=True)

    with tile.TileContext(nc) as tc:
        with tc.tile_pool(name="const", bufs=1) as const:
            id_sb = const.tile([T, T], BF16, name="id_sb")
            make_identity(nc, id_sb)

            # small input DMAs ride the gpsimd queue so the first weight
            # slab heads the sync/HWDGE ring with nothing in front of it
            xrw_sb = const.tile([P, KH * XW], F32, name="xrw_sb")
            nc.gpsimd.dma_start(out=xrw_sb, in_=xrw_d[:, :])
            xtb_sb = const.tile([P, KH * T], BF16, name="xtb_sb")
            nc.gpsimd.dma_start(out=xtb_sb, in_=xtb_d[:, :])

            interT_sb = const.tile([P, KI * T], BF16, name="interT_sb")
            out_sb = const.tile([T, H], F32, name="out_sb")
            comb_sb = const.tile([T, 1], F32, name="comb_sb")

            def xT_k(k):  # [128, 32] stationary bf16 activation chunk
                return xtb_sb[:, k * T : (k + 1) * T]

            def xf_k(k):  # [128, 32] fp32 activation chunk (router)
                return xrw_sb[:, k * XW : k * XW + T]

            def rw_k(k):  # [128, 8] fp32 router weight chunk
                return xrw_sb[:, k * XW + T : (k + 1) * XW]

            wgp = tc.alloc_tile_pool(name="wgp", bufs=3)
            wup = tc.alloc_tile_pool(name="wup", bufs=3)
            wdp = tc.alloc_tile_pool(name="wdp", bufs=2)

            # ---------------- router: softmax + top-2 mask ----------------
            with (
                tc.tile_pool(name="rps", bufs=1, space="PSUM") as rps,
                tc.tile_pool(name="rsb", bufs=1) as rsb,
            ):
                # absorb the ident DMA tick on PE before anything else
                dmy_ps = rps.tile([T, T], BF16, name="dmy_ps", tag="dmy")
                nc.tensor.transpose(dmy_ps, id_sb, id_sb)

                logits = rps.tile([T, E], F32, name="logits", tag="logits")
                for k in range(KH):
                    nc.tensor.matmul(
                        logits,
                        xf_k(k),
                        rw_k(k),
                        start=(k == 0),
                        stop=(k == KH - 1),
                    )
                # PSUM is read only by DVE (keeps later PE writers 1-wait)
                lg = rsb.tile([T, E], F32, name="lg")
                nc.vector.tensor_copy(lg, logits)
                mx = rsb.tile([T, 1], F32, name="mx")
                nc.vector.reduce_max(mx, lg, axis=AX)
                nmx = rsb.tile([T, 1], F32, name="nmx")
                nc.vector.tensor_scalar_mul(nmx, mx, -1.0)
                ex = rsb.tile([T, E], F32, name="ex")
                nc.scalar.activation(ex, lg, AF.Exp, bias=nmx, scale=1.0)
                sm = rsb.tile([T, 1], F32, name="sm")
                nc.vector.reduce_sum(sm, ex, axis=AX)
                rc = rsb.tile([T, 1], F32, name="rc")
                nc.vector.reciprocal(rc, sm)
                aff = rsb.tile([T, E], F32, name="aff")
                nc.vector.tensor_scalar_mul(aff, ex, rc)
                # top-2: value >= (second largest)
                m1 = rsb.tile([T, 1], F32, name="m1")
                nc.vector.reduce_max(m1, aff, axis=AX)
                pen = rsb.tile([T, E], F32, name="pen")
                nc.vector.tensor_scalar(
                    pen, aff, m1, -1e30, op0=OP.is_equal, op1=OP.mult
                )
                b2 = rsb.tile([T, E], F32, name="b2")
                nc.vector.tensor_add(b2, aff, pen)
                m2 = rsb.tile([T, 1], F32, name="m2")
                nc.vector.reduce_max(m2, b2, axis=AX)
                ge = rsb.tile([T, E], F32, name="ge")
                nc.vector.tensor_scalar(ge, aff, m2, None, op0=OP.is_ge)
                msk = rsb.tile([T, E], F32, name="msk")
                nc.vector.tensor_mul(msk, aff, ge)
                # rolled router weight puts the own expert at column 0
                nc.vector.tensor_copy(comb_sb, msk[:, 0:1])

            # ---- fused gate/up + swiglu + transpose + interleaved down ----
            # Down matmuls run ONE SLAB BEHIND gate/up: while slab n's
            # epilogue runs on DVE/ACT, the PE streams slab n-1's down
            # chunks instead of idling on the gate/up PSUM (bufs=1) - the
            # PE order per slab is [gate(n), up(n), down(n-1), transp(n)].
            # wd streams through the whole kernel on the gpsimd queue.
            # PSUM: gate/up 2 + transpose 2 + down accumulators 4 = 8 banks.
            with (
                tc.tile_pool(name="gup", bufs=1, space="PSUM") as gup,
                tc.tile_pool(name="tps", bufs=2, space="PSUM") as tps,
                tc.tile_pool(name="dps", bufs=1, space="PSUM") as dps,
                tc.tile_pool(name="esb", bufs=2) as esb,
            ):
                d_ps = [
                    dps.tile([T, NW], F32, name=f"d_ps{j}", tag=f"d{j}")
                    for j in range(ND)
                ]

                def down_block(ki0, kis, wd_sl):
                    for dki in range(kis):
                        ki = ki0 + dki
                        for j in range(ND):
                            nc.tensor.matmul(
                                d_ps[j],
                                interT_sb[:, ki * T : (ki + 1) * T],
                                wd_sl[:, dki * H + j * NW : dki * H + (j + 1) * NW],
                                start=(ki == 0),
                                stop=(ki == KI - 1),
                            )

                prev_down = None
                c0 = 0
                for n, w in enumerate(WIDTHS):
                    wg_sl = wgp.tile([P, KH * NW], BF16, name="wg_sl", tag="wg")
                    wu_sl = wup.tile([P, KH * NW], BF16, name="wu_sl", tag="wu")
                    # k-halves so matmuls overlap each slab's weight stream
                    half = (KH // 2) * w
                    for hb in range(2):
                        nc.sync.dma_start(
                            out=wg_sl[:, hb * half : (hb + 1) * half],
                            in_=wg_d[
                                :, c0 * KH + hb * half : c0 * KH + (hb + 1) * half
                            ],
                        )
                    for hb in range(2):
                        nc.sync.dma_start(
                            out=wu_sl[:, hb * half : (hb + 1) * half],
                            in_=wu_d[
                                :, c0 * KH + hb * half : c0 * KH + (hb + 1) * half
                            ],
                        )
                    g_ps = gup.tile([T, NW], F32, name="g_ps", tag="g")
                    u_ps = gup.tile([T, NW], F32, name="u_ps", tag="u")
                    for k in range(KH):
                        nc.tensor.matmul(
                            g_ps[:, :w],
                            xT_k(k),
                            wg_sl[:, k * w : (k + 1) * w],
                            start=(k == 0),
                            stop=(k == KH - 1),
                        )
                    for k in range(KH):
                        nc.tensor.matmul(
                            u_ps[:, :w],
                            xT_k(k),
                            wu_sl[:, k * w : (k + 1) * w],
                            start=(k == 0),
                            stop=(k == KH - 1),
                        )
                    # fetch this slab's wd chunk; consumed next iteration
                    ki0 = c0 // P
                    kis = w // P
                    wd_sl = wdp.tile([P, (NW // P) * H], BF16, name="wd_sl", tag="wd")
                    nc.gpsimd.dma_start(
                        out=wd_sl[:, : kis * H],
                        in_=wd_d[:, ki0 * H : (ki0 + kis) * H],
                    )
                    # PE fills the epilogue latency with slab n-1's down MMs
                    if prev_down is not None:
                        down_block(*prev_down)
                    prev_down = (ki0, kis, wd_sl)
                    # epilogue: PSUM read only by DVE; sigmoid runs off a copy
                    g_sb = esb.tile([T, NW], F32, name="g_sb", tag="gsb")
                    nc.vector.tensor_copy(g_sb[:, :w], g_ps[:, :w])
                    sig = esb.tile([T, NW], F32, name="sig", tag="sig")
                    nc.scalar.activation(
                        sig[:, :w], g_sb[:, :w], AF.Sigmoid, scale=SWIGLU_SCALE
                    )
                    t1 = esb.tile([T, NW], F32, name="t1", tag="t1")
                    nc.vector.tensor_mul(t1[:, :w], g_ps[:, :w], sig[:, :w])
                    inter = esb.tile([T, NW], BF16, name="inter", tag="inter")
                    nc.vector.tensor_mul(inter[:, :w], t1[:, :w], u_ps[:, :w])
                    for j in range(w // P):
                        ic = c0 // P + j
                        tp = tps.tile([P, T], BF16, name="tp", tag="tp")
                        nc.tensor.transpose(tp, inter[:, j * P : (j + 1) * P], id_sb)
                        nc.vector.tensor_copy(
                            interT_sb[:, ic * T : (ic + 1) * T], tp
                        )
                    c0 += w
                down_block(*prev_down)
                for j in range(ND):
                    nc.vector.tensor_scalar_mul(
                        out_sb[:, j * NW : (j + 1) * NW], d_ps[j], comb_sb
                    )
                    nc.sync.dma_start(
                        out=out_d[:, j * NW : (j + 1) * NW],
                        in_=out_sb[:, j * NW : (j + 1) * NW],
                    )
            wdp.release()
            wup.release()
            wgp.release()
    nc.finalize()
    return nc


def _block_rows(a: np.ndarray) -> np.ndarray:
    """[R, C] row-major -> [P, (R//P)*C]; partition p holds rows k*P+p
    as contiguous (k, c) runs, matching SBUF tiles sliced per k-chunk."""
    Rr, C = a.shape
    return np.ascontiguousarray(
        a.reshape(Rr // P, P, C).transpose(1, 0, 2).reshape(P, (Rr // P) * C)
    )


def _make_in_maps(hidden_states, router_weight, w_gate, w_up, w_down):
    BF = ml_dtypes.bfloat16
    x = np.ascontiguousarray(np.asarray(hidden_states, np.float32).reshape(T, H))
    rw = np.asarray(router_weight, np.float32)
    wg = np.asarray(w_gate, np.float32).astype(BF)
    wu = np.asarray(w_up, np.float32).astype(BF)
    wd = np.asarray(w_down, np.float32).astype(BF)
    xT = np.ascontiguousarray(x.T)  # [H, T] fp32 (router accuracy)
    xtb = _block_rows(xT.astype(BF))  # [P, KH*T] bf16

    in_maps = []
    for c in range(NCORES):
        order = [(j + c) % E for j in range(E)]  # column j holds expert (j+c)%E
        rwT = rw[order].T  # [H, E]; col 0 = own expert
        xrw = _block_rows(
            np.ascontiguousarray(np.concatenate([xT, rwT], axis=1))
        )  # [P, KH*(T+E)]

        # gate/up: blocked per column-slab so each slab DMA is contiguous
        arr_g = wg[c].reshape(KH, P, I).transpose(1, 0, 2)  # [P, KH, I]
        arr_u = wu[c].reshape(KH, P, I).transpose(1, 0, 2)
        gs, us, c0 = [], [], 0
        for w in WIDTHS:
            gs.append(arr_g[:, :, c0 : c0 + w].reshape(P, KH * w))
            us.append(arr_u[:, :, c0 : c0 + w].reshape(P, KH * w))
            c0 += w
        wg_b = np.ascontiguousarray(np.concatenate(gs, axis=1))
        wu_b = np.ascontiguousarray(np.concatenate(us, axis=1))
        wd_b = _block_rows(wd[c])  # [P, KI*H]

        in_maps.append(
            {
                "xrw": xrw,
                "xtb": xtb,
                "wg": wg_b,
                "wu": wu_b,
                "wd": wd_b,
            }
        )
    return in_maps


def kernel(
    hidden_states,
    router_weight,
    w_gate,
    w_up,
    w_down,
    top_k,
    _trace: bool = False,
    _trace_all: bool = False,
):
    assert int(top_k) == 2, "kernel hardcodes top_k=2"
    in_maps = _make_in_maps(hidden_states, router_weight, w_gate, w_up, w_down)
    nc = _build_nc()
    res = run_bass_kernel_spmd(
        nc, in_maps, core_ids=list(range(NCORES)), trace=_trace,
        trace_cores=list(range(NCORES)) if (_trace and _trace_all) else None,
    )
    outs = np.stack([res.results[c]["out"] for c in range(NCORES)], axis=0)
    out = outs.sum(axis=0, dtype=np.float64).astype(np.float32)
    if _trace:
        kernel.last_exec_time_ns = res.exec_time_ns
        kernel.last_mean_exec_time_ns = res.mean_exec_time_ns
        kernel.last_trace = res.instructions_and_trace
    return out.reshape(B, S, H)
